# revision 16
# baseline (speedup 1.0000x reference)
"""AELoss (associative embedding push/pull loss) on 8 TRN2 NeuronCores.

Strategy: pure data parallel. B=32 images, 4 per core. The reference only
ever *reads* P*J = 510 scalar tag values per image out of the 1.1M-entry tag
map, so instead of streaming the whole 17.8MB/core tags shard (~45us at HBM
bandwidth) we gather exactly the 2040 needed scalars per core with
gpsimd indirect DMAs, and the whole loss then fits in a single [120, *]
SBUF tile (120 = 4 images x 30 people on partitions, joints on the free
dim).

Gather: the HW indirect DMA consumes ONE offset per partition-row
descriptor (and would copy dest-row-length contiguous elements), so the
[120,17] gather is issued as 17 single-column indirect DMAs of 120
descriptors each. Each instruction costs ~1.1us of Q7 descriptor-gen time
(SWDGE fixed overhead) and they serialize on the GpSimd engine — this is
the dominant cost. (The batched Q7 GATHER ucode / tensor_tensor_reduce
paths fault on this runtime, so this is the fastest working form.)

The host pre-folds the image offset b*N into the joint indices while
slicing the shard (index into the flat [4N] shard), so the gathers issue as
soon as the single small joints DMA lands. Per-person stats use the
algebraic pull form sum(g^2 v)/cnt - mean^2; per-image reductions over the
30 person-partitions are one PE matmul against a block-column selector; the
pairwise push mask valid[p]*valid[f]*sameimg[p,f] is one K=4 matmul
vb @ vb^T with vb[p,b] = valid[p] * (p in image b), and exp(-diff^2) is a
single [120,120] ACT pass.
"""

import numpy as np

B = 32
N = 17 * 256 * 256  # 1114112 flattened tag-map size
P = 30              # max people per image
J = 17              # joints per person
M = 8               # cores
BL = B // M         # images per core = 4
PART = BL * P       # partitions used = 120

_CACHE = {}


def _build():
    from contextlib import ExitStack

    import concourse.bass as bass
    import concourse.tile as tile
    from concourse import bacc, mybir
    from concourse.masks import make_identity

    f32 = mybir.dt.float32
    i32 = mybir.dt.int32
    Alu = mybir.AluOpType
    X = mybir.AxisListType.X

    nc = bacc.Bacc("TRN2", target_bir_lowering=False, debug=False)

    tags = nc.dram_tensor("tags", [BL * N, 1], f32, kind="ExternalInput")
    jnt = nc.dram_tensor("jnt", [PART, 2 * J], i32, kind="ExternalInput")
    out = nc.dram_tensor("out", [BL, 2], f32, kind="ExternalOutput")

    with tile.TileContext(nc) as tc:
        with ExitStack() as ctx:
            sb = ctx.enter_context(tc.tile_pool(name="sb", bufs=1))
            ps = ctx.enter_context(tc.tile_pool(name="ps", bufs=1, space="PSUM"))

            jnt_t = sb.tile([PART, 2 * J], i32)
            nc.sync.dma_start(out=jnt_t[:], in_=jnt[:, :])
            ji = jnt_t[:, 0:J]
            jv = jnt_t[:, J:2 * J]

            # gather g[p, j] = tags[ji[p, j]] (offsets pre-folded on host).
            # HW indirect DMA consumes one offset per partition-row
            # descriptor, so one DMA per joint column.
            g = sb.tile([PART, J], f32)
            for j in range(J):
                nc.gpsimd.indirect_dma_start(
                    out=g[:, j:j + 1],
                    out_offset=None,
                    in_=tags[:, :],
                    in_offset=bass.IndirectOffsetOnAxis(ap=ji[:, j:j + 1], axis=0),
                )

            # ---- off-critical-path prep (overlaps the gathers) ----
            # s[p] = p // 30 = image id (3 threshold sums), sel[p,b] one-hot
            ptile = sb.tile([PART, 1], i32)
            nc.gpsimd.iota(out=ptile[:], pattern=[[0, 1]], base=0,
                           channel_multiplier=1)
            s = sb.tile([PART, 1], i32)
            t60 = sb.tile([PART, 1], i32)
            t90 = sb.tile([PART, 1], i32)
            nc.vector.tensor_scalar(out=s[:], in0=ptile[:], scalar1=P,
                                    scalar2=None, op0=Alu.is_ge)
            nc.vector.tensor_scalar(out=t60[:], in0=ptile[:], scalar1=2 * P,
                                    scalar2=None, op0=Alu.is_ge)
            nc.vector.tensor_scalar(out=t90[:], in0=ptile[:], scalar1=3 * P,
                                    scalar2=None, op0=Alu.is_ge)
            nc.vector.tensor_tensor(out=s[:], in0=s[:], in1=t60[:], op=Alu.add)
            nc.vector.tensor_tensor(out=s[:], in0=s[:], in1=t90[:], op=Alu.add)
            biota = sb.tile([PART, BL], i32)
            nc.gpsimd.iota(out=biota[:], pattern=[[1, BL]], base=0,
                           channel_multiplier=0)
            sel = sb.tile([PART, BL], f32)
            nc.vector.tensor_tensor(out=sel[:], in0=s[:].to_broadcast([PART, BL]),
                                    in1=biota[:], op=Alu.is_equal)

            ident = sb.tile([128, 128], f32)
            make_identity(nc, ident[:])

            # visibility stats (don't need g): visf + cnt fused
            visf = sb.tile([PART, J], f32)
            cnt = sb.tile([PART, 1], f32)
            nc.vector.tensor_scalar(out=visf[:], in0=jv, scalar1=0,
                                    scalar2=None, op0=Alu.is_gt)
            nc.vector.reduce_sum(out=cnt[:], in_=visf[:], axis=X)
            safe_cnt = sb.tile([PART, 1], f32)
            nc.vector.tensor_scalar_max(out=safe_cnt[:], in0=cnt[:], scalar1=1.0)
            icnt = sb.tile([PART, 1], f32)
            nc.vector.reciprocal(out=icnt[:], in_=safe_cnt[:])

            # stacked columns: 0 = pull_p*valid, 1 = valid, 2 = push row sums
            stacked = sb.tile([PART, 3], f32)
            nc.vector.tensor_scalar(out=stacked[:, 1:2], in0=cnt[:], scalar1=0.0,
                                    scalar2=None, op0=Alu.is_gt)

            # vb[p, b] = valid[p] * sel[p, b]; pairmask = vb @ vb^T via PE
            vb = sb.tile([PART, BL], f32)
            nc.vector.tensor_scalar(out=vb[:], in0=sel[:],
                                    scalar1=stacked[:, 1:2], scalar2=None,
                                    op0=Alu.mult)
            vbT_ps = ps.tile([BL, PART], f32, space="PSUM")
            nc.tensor.transpose(out=vbT_ps[:], in_=vb[:],
                                identity=ident[:PART, :PART])
            vbT = sb.tile([BL, PART], f32)
            nc.vector.tensor_copy(out=vbT[:], in_=vbT_ps[:])
            mask_ps = ps.tile([PART, PART], f32, space="PSUM")
            nc.tensor.matmul(out=mask_ps[:], lhsT=vbT[:], rhs=vbT[:],
                             start=True, stop=True)

            # ---- g-dependent chain ----
            # gv = g*visf with accum sum_gv; then sum_g2v = sum(gv*g)
            # split stats into column halves: the a-half (cols 0:9) only
            # depends on the first 9 gathers, so it runs under the shadow of
            # gathers 9..16; only the b-half trails the last gather.
            JA = 9
            gv = sb.tile([PART, J], f32)
            scrap = sb.tile([PART, J], f32)
            sgv2 = sb.tile([PART, 2], f32)
            sggv2 = sb.tile([PART, 2], f32)
            nc.vector.tensor_tensor(out=gv[:, :JA], in0=g[:, :JA],
                                    in1=visf[:, :JA], op=Alu.mult)
            nc.vector.reduce_sum(out=sgv2[:, 0:1], in_=gv[:, :JA], axis=X)
            nc.vector.tensor_tensor(out=scrap[:, :JA], in0=gv[:, :JA],
                                    in1=g[:, :JA], op=Alu.mult)
            nc.vector.reduce_sum(out=sggv2[:, 0:1], in_=scrap[:, :JA], axis=X)
            nc.vector.tensor_tensor(out=gv[:, JA:], in0=g[:, JA:],
                                    in1=visf[:, JA:], op=Alu.mult)
            nc.vector.reduce_sum(out=sgv2[:, 1:2], in_=gv[:, JA:], axis=X)
            nc.vector.tensor_tensor(out=scrap[:, JA:], in0=gv[:, JA:],
                                    in1=g[:, JA:], op=Alu.mult)
            nc.vector.reduce_sum(out=sggv2[:, 1:2], in_=scrap[:, JA:], axis=X)
            sgv = sb.tile([PART, 1], f32)
            nc.vector.tensor_tensor(out=sgv[:], in0=sgv2[:, 0:1],
                                    in1=sgv2[:, 1:2], op=Alu.add)
            sggv = sb.tile([PART, 1], f32)
            nc.vector.tensor_tensor(out=sggv[:], in0=sggv2[:, 0:1],
                                    in1=sggv2[:, 1:2], op=Alu.add)
            mean = sb.tile([PART, 1], f32)
            nc.vector.tensor_tensor(out=mean[:], in0=sgv[:], in1=icnt[:],
                                    op=Alu.mult)
            # pull_p*valid = (sum_g2v*icnt - mean^2) * valid
            a2 = sb.tile([PART, 1], f32)
            nc.vector.tensor_tensor(out=a2[:], in0=sggv[:], in1=icnt[:],
                                    op=Alu.mult)
            mean2 = sb.tile([PART, 1], f32)
            nc.vector.tensor_tensor(out=mean2[:], in0=mean[:], in1=mean[:],
                                    op=Alu.mult)
            nc.vector.scalar_tensor_tensor(out=stacked[:, 0:1], in0=a2[:],
                                           scalar=mean2[:], in1=stacked[:, 1:2],
                                           op0=Alu.subtract, op1=Alu.mult)

            # pairwise push field over the person-partitions
            meanT = ps.tile([PART, PART], f32, space="PSUM")
            nc.tensor.transpose(out=meanT[:],
                                in_=mean[:].to_broadcast([PART, PART]),
                                identity=ident[:PART, :PART])
            diff = sb.tile([PART, PART], f32)
            nc.vector.tensor_tensor(out=diff[:],
                                    in0=mean[:].to_broadcast([PART, PART]),
                                    in1=meanT[:], op=Alu.subtract)
            sq = sb.tile([PART, PART], f32)
            nc.vector.tensor_tensor(out=sq[:], in0=diff[:], in1=diff[:],
                                    op=Alu.mult)
            pm = sb.tile([PART, PART], f32)
            nc.scalar.activation(out=pm[:], in_=sq[:],
                                 func=mybir.ActivationFunctionType.Exp,
                                 scale=-1.0)
            pmm = sb.tile([PART, PART], f32)
            nc.vector.tensor_tensor(out=pmm[:], in0=pm[:], in1=mask_ps[:],
                                    op=Alu.mult)
            nc.vector.reduce_sum(out=stacked[:, 2:3], in_=pmm[:], axis=X)

            # per-image reduction over person-partitions
            red = ps.tile([BL, 3], f32, space="PSUM")
            nc.tensor.matmul(out=red[:], lhsT=sel[:], rhs=stacked[:],
                             start=True, stop=True)
            # red columns: 0 = pull_sum, 1 = num_tags, 2 = raw push sum
            reds = sb.tile([BL, 3], f32)
            nc.vector.tensor_copy(out=reds[:], in_=red[:])

            outt = sb.tile([BL, 2], f32)
            s_nt = sb.tile([BL, 1], f32)
            nc.vector.tensor_scalar_max(out=s_nt[:], in0=reds[:, 1:2], scalar1=1.0)
            inv_nt = sb.tile([BL, 1], f32)
            nc.vector.reciprocal(out=inv_nt[:], in_=s_nt[:])
            nc.vector.tensor_tensor(out=outt[:, 1:2], in0=reds[:, 0:1],
                                    in1=inv_nt[:], op=Alu.mult)

            psub = sb.tile([BL, 1], f32)
            nc.vector.tensor_tensor(out=psub[:], in0=reds[:, 2:3],
                                    in1=reds[:, 1:2], op=Alu.subtract)
            den = sb.tile([BL, 1], f32)
            nc.vector.scalar_tensor_tensor(out=den[:], in0=reds[:, 1:2],
                                           scalar=1.0, in1=reds[:, 1:2],
                                           op0=Alu.subtract, op1=Alu.mult)
            nc.vector.tensor_scalar_max(out=den[:], in0=den[:], scalar1=1.0)
            invden = sb.tile([BL, 1], f32)
            nc.vector.reciprocal(out=invden[:], in_=den[:])
            half = sb.tile([BL, 1], f32)
            nc.vector.scalar_tensor_tensor(out=half[:], in0=psub[:],
                                           scalar=0.5, in1=invden[:],
                                           op0=Alu.mult, op1=Alu.mult)
            gate = sb.tile([BL, 1], f32)
            nc.vector.tensor_scalar(out=gate[:], in0=reds[:, 1:2], scalar1=1.0,
                                    scalar2=None, op0=Alu.is_gt)
            nc.vector.tensor_tensor(out=outt[:, 0:1], in0=half[:], in1=gate[:],
                                    op=Alu.mult)

            nc.sync.dma_start(out=out[:, :], in_=outt[:])

    nc.compile()
    return nc


def _get_nc():
    if "nc" not in _CACHE:
        _CACHE["nc"] = _build()
    return _CACHE["nc"]


def _make_in_maps(tags: np.ndarray, joints: np.ndarray):
    tags = np.asarray(tags, dtype=np.float32).reshape(B, N)
    joints = np.asarray(joints, dtype=np.int32)
    boffs = (np.arange(BL, dtype=np.int32) * N)[:, None, None]
    in_maps = []
    for i in range(M):
        t = np.ascontiguousarray(tags[i * BL:(i + 1) * BL].reshape(BL * N, 1))
        sl = joints[i * BL:(i + 1) * BL]  # [BL, P, J, 2]
        ji = (sl[..., 0] + boffs).reshape(PART, J)
        jv = sl[..., 1].reshape(PART, J)
        jnt = np.ascontiguousarray(np.concatenate([ji, jv], axis=1))
        in_maps.append({"tags": t, "jnt": jnt})
    return in_maps


def _run(tags, joints, trace=False):
    from concourse.bass_utils import run_bass_kernel_spmd

    nc = _get_nc()
    in_maps = _make_in_maps(tags, joints)
    res = run_bass_kernel_spmd(
        nc, in_maps, core_ids=list(range(M)), trace=trace,
    )
    outs = [res.results[i]["out"] for i in range(M)]
    push = np.concatenate([o[:, 0] for o in outs]).astype(np.float32)
    pull = np.concatenate([o[:, 1] for o in outs]).astype(np.float32)
    return (push, pull), res.exec_time_ns


def kernel(tags, joints):
    (push, pull), _ = _run(tags, joints, trace=False)
    return push, pull


# revision 17
# speedup vs baseline: 1.0013x; 1.0013x over previous
"""AELoss (associative embedding push/pull loss) on 8 TRN2 NeuronCores.

Strategy: pure data parallel. B=32 images, 4 per core. The reference only
ever *reads* P*J = 510 scalar tag values per image out of the 1.1M-entry tag
map, so instead of streaming the whole 17.8MB/core tags shard (~45us at HBM
bandwidth) we gather exactly the 2040 needed scalars per core with
gpsimd indirect DMAs, and the whole loss then fits in a single [120, *]
SBUF tile (120 = 4 images x 30 people on partitions, joints on the free
dim).

Gather: the HW indirect DMA consumes ONE offset per partition-row
descriptor (and would copy dest-row-length contiguous elements), so the
[120,17] gather is issued as 17 single-column indirect DMAs of 120
descriptors each. Each instruction costs ~1.1us of Q7 descriptor-gen time
(SWDGE fixed overhead) and they serialize on the GpSimd engine — this is
the dominant cost. (The batched Q7 GATHER ucode / tensor_tensor_reduce
paths fault on this runtime, so this is the fastest working form.)

The host pre-folds the image offset b*N into the joint indices while
slicing the shard (index into the flat [4N] shard), so the gathers issue as
soon as the single small joints DMA lands. Per-person stats use the
algebraic pull form sum(g^2 v)/cnt - mean^2; per-image reductions over the
30 person-partitions are one PE matmul against a block-column selector; the
pairwise push mask valid[p]*valid[f]*sameimg[p,f] is one K=4 matmul
vb @ vb^T with vb[p,b] = valid[p] * (p in image b), and exp(-diff^2) is a
single [120,120] ACT pass.
"""

import numpy as np

B = 32
N = 17 * 256 * 256  # 1114112 flattened tag-map size
P = 30              # max people per image
J = 17              # joints per person
M = 8               # cores
BL = B // M         # images per core = 4
PART = BL * P       # partitions used = 120

_CACHE = {}


def _build():
    from contextlib import ExitStack

    import concourse.bass as bass
    import concourse.tile as tile
    from concourse import bacc, mybir
    from concourse.masks import make_identity

    f32 = mybir.dt.float32
    i32 = mybir.dt.int32
    Alu = mybir.AluOpType
    X = mybir.AxisListType.X

    nc = bacc.Bacc("TRN2", target_bir_lowering=False, debug=False)

    tags = nc.dram_tensor("tags", [BL * N, 1], f32, kind="ExternalInput")
    jnt = nc.dram_tensor("jnt", [PART, 2 * J], i32, kind="ExternalInput")
    out = nc.dram_tensor("out", [BL, 2], f32, kind="ExternalOutput")

    with tile.TileContext(nc) as tc:
        with ExitStack() as ctx:
            sb = ctx.enter_context(tc.tile_pool(name="sb", bufs=1))
            ps = ctx.enter_context(tc.tile_pool(name="ps", bufs=1, space="PSUM"))

            jnt_t = sb.tile([PART, 2 * J], i32)
            nc.sync.dma_start(out=jnt_t[:], in_=jnt[:, :])
            ji = jnt_t[:, 0:J]
            jv = jnt_t[:, J:2 * J]

            # gather g[p, j] = tags[ji[p, j]] (offsets pre-folded on host).
            # HW indirect DMA consumes one offset per partition-row
            # descriptor, so one DMA per joint column.
            g = sb.tile([PART, J], f32)
            for j in range(J):
                nc.gpsimd.indirect_dma_start(
                    out=g[:, j:j + 1],
                    out_offset=None,
                    in_=tags[:, :],
                    in_offset=bass.IndirectOffsetOnAxis(ap=ji[:, j:j + 1], axis=0),
                )

            # ---- off-critical-path prep (overlaps the gathers) ----
            # s[p] = p // 30 = image id (3 threshold sums), sel[p,b] one-hot
            ptile = sb.tile([PART, 1], i32)
            nc.gpsimd.iota(out=ptile[:], pattern=[[0, 1]], base=0,
                           channel_multiplier=1)
            s = sb.tile([PART, 1], i32)
            t60 = sb.tile([PART, 1], i32)
            t90 = sb.tile([PART, 1], i32)
            nc.vector.tensor_scalar(out=s[:], in0=ptile[:], scalar1=P,
                                    scalar2=None, op0=Alu.is_ge)
            nc.vector.tensor_scalar(out=t60[:], in0=ptile[:], scalar1=2 * P,
                                    scalar2=None, op0=Alu.is_ge)
            nc.vector.tensor_scalar(out=t90[:], in0=ptile[:], scalar1=3 * P,
                                    scalar2=None, op0=Alu.is_ge)
            nc.vector.tensor_tensor(out=s[:], in0=s[:], in1=t60[:], op=Alu.add)
            nc.vector.tensor_tensor(out=s[:], in0=s[:], in1=t90[:], op=Alu.add)
            biota = sb.tile([PART, BL], i32)
            nc.gpsimd.iota(out=biota[:], pattern=[[1, BL]], base=0,
                           channel_multiplier=0)
            sel = sb.tile([PART, BL], f32)
            nc.vector.tensor_tensor(out=sel[:], in0=s[:].to_broadcast([PART, BL]),
                                    in1=biota[:], op=Alu.is_equal)

            ident = sb.tile([128, 128], f32)
            make_identity(nc, ident[:])

            # visibility stats (don't need g): visf + cnt fused
            visf = sb.tile([PART, J], f32)
            cnt = sb.tile([PART, 1], f32)
            nc.vector.tensor_scalar(out=visf[:], in0=jv, scalar1=0,
                                    scalar2=None, op0=Alu.is_gt)
            nc.vector.reduce_sum(out=cnt[:], in_=visf[:], axis=X)
            safe_cnt = sb.tile([PART, 1], f32)
            nc.vector.tensor_scalar_max(out=safe_cnt[:], in0=cnt[:], scalar1=1.0)
            icnt = sb.tile([PART, 1], f32)
            nc.vector.reciprocal(out=icnt[:], in_=safe_cnt[:])

            # stacked columns: 0 = pull_p*valid, 1 = valid, 2 = push row sums
            stacked = sb.tile([PART, 3], f32)
            nc.vector.tensor_scalar(out=stacked[:, 1:2], in0=cnt[:], scalar1=0.0,
                                    scalar2=None, op0=Alu.is_gt)

            # vb[p, b] = valid[p] * sel[p, b]; pairmask = vb @ vb^T via PE
            vb = sb.tile([PART, BL], f32)
            nc.vector.tensor_scalar(out=vb[:], in0=sel[:],
                                    scalar1=stacked[:, 1:2], scalar2=None,
                                    op0=Alu.mult)
            vbT_ps = ps.tile([BL, PART], f32, space="PSUM")
            nc.tensor.transpose(out=vbT_ps[:], in_=vb[:],
                                identity=ident[:PART, :PART])
            vbT = sb.tile([BL, PART], f32)
            nc.vector.tensor_copy(out=vbT[:], in_=vbT_ps[:])
            mask_ps = ps.tile([PART, PART], f32, space="PSUM")
            nc.tensor.matmul(out=mask_ps[:], lhsT=vbT[:], rhs=vbT[:],
                             start=True, stop=True)

            # ---- g-dependent chain ----
            # gv = g*visf with accum sum_gv; then sum_g2v = sum(gv*g)
            gv = sb.tile([PART, J], f32)
            sgv = sb.tile([PART, 1], f32)
            nc.vector.tensor_tensor(out=gv[:], in0=g[:], in1=visf[:], op=Alu.mult)
            nc.vector.reduce_sum(out=sgv[:], in_=gv[:], axis=X)
            scrap = sb.tile([PART, J], f32)
            sggv = sb.tile([PART, 1], f32)
            nc.vector.tensor_tensor(out=scrap[:], in0=gv[:], in1=g[:], op=Alu.mult)
            nc.vector.reduce_sum(out=sggv[:], in_=scrap[:], axis=X)
            mean = sb.tile([PART, 1], f32)
            nc.vector.tensor_tensor(out=mean[:], in0=sgv[:], in1=icnt[:],
                                    op=Alu.mult)
            # pull_p*valid = (sum_g2v*icnt - mean^2) * valid
            a2 = sb.tile([PART, 1], f32)
            nc.vector.tensor_tensor(out=a2[:], in0=sggv[:], in1=icnt[:],
                                    op=Alu.mult)
            mean2 = sb.tile([PART, 1], f32)
            nc.vector.tensor_tensor(out=mean2[:], in0=mean[:], in1=mean[:],
                                    op=Alu.mult)
            nc.vector.scalar_tensor_tensor(out=stacked[:, 0:1], in0=a2[:],
                                           scalar=mean2[:], in1=stacked[:, 1:2],
                                           op0=Alu.subtract, op1=Alu.mult)

            # pairwise push field over the person-partitions
            meanT = ps.tile([PART, PART], f32, space="PSUM")
            nc.tensor.transpose(out=meanT[:],
                                in_=mean[:].to_broadcast([PART, PART]),
                                identity=ident[:PART, :PART])
            diff = sb.tile([PART, PART], f32)
            nc.vector.tensor_tensor(out=diff[:],
                                    in0=mean[:].to_broadcast([PART, PART]),
                                    in1=meanT[:], op=Alu.subtract)
            sq = sb.tile([PART, PART], f32)
            nc.vector.tensor_tensor(out=sq[:], in0=diff[:], in1=diff[:],
                                    op=Alu.mult)
            pm = sb.tile([PART, PART], f32)
            nc.scalar.activation(out=pm[:], in_=sq[:],
                                 func=mybir.ActivationFunctionType.Exp,
                                 scale=-1.0)
            pmm = sb.tile([PART, PART], f32)
            nc.vector.tensor_tensor(out=pmm[:], in0=pm[:], in1=mask_ps[:],
                                    op=Alu.mult)
            nc.vector.reduce_sum(out=stacked[:, 2:3], in_=pmm[:], axis=X)

            # per-image reduction over person-partitions
            red = ps.tile([BL, 3], f32, space="PSUM")
            nc.tensor.matmul(out=red[:], lhsT=sel[:], rhs=stacked[:],
                             start=True, stop=True)
            # red columns: 0 = pull_sum, 1 = num_tags, 2 = raw push sum
            reds = sb.tile([BL, 3], f32)
            nc.vector.tensor_copy(out=reds[:], in_=red[:])

            outt = sb.tile([BL, 2], f32)
            s_nt = sb.tile([BL, 1], f32)
            nc.vector.tensor_scalar_max(out=s_nt[:], in0=reds[:, 1:2], scalar1=1.0)
            inv_nt = sb.tile([BL, 1], f32)
            nc.vector.reciprocal(out=inv_nt[:], in_=s_nt[:])
            nc.vector.tensor_tensor(out=outt[:, 1:2], in0=reds[:, 0:1],
                                    in1=inv_nt[:], op=Alu.mult)

            psub = sb.tile([BL, 1], f32)
            nc.vector.tensor_tensor(out=psub[:], in0=reds[:, 2:3],
                                    in1=reds[:, 1:2], op=Alu.subtract)
            den = sb.tile([BL, 1], f32)
            nc.vector.scalar_tensor_tensor(out=den[:], in0=reds[:, 1:2],
                                           scalar=1.0, in1=reds[:, 1:2],
                                           op0=Alu.subtract, op1=Alu.mult)
            nc.vector.tensor_scalar_max(out=den[:], in0=den[:], scalar1=1.0)
            invden = sb.tile([BL, 1], f32)
            nc.vector.reciprocal(out=invden[:], in_=den[:])
            half = sb.tile([BL, 1], f32)
            nc.vector.scalar_tensor_tensor(out=half[:], in0=psub[:],
                                           scalar=0.5, in1=invden[:],
                                           op0=Alu.mult, op1=Alu.mult)
            gate = sb.tile([BL, 1], f32)
            nc.vector.tensor_scalar(out=gate[:], in0=reds[:, 1:2], scalar1=1.0,
                                    scalar2=None, op0=Alu.is_gt)
            nc.vector.tensor_tensor(out=outt[:, 0:1], in0=half[:], in1=gate[:],
                                    op=Alu.mult)

            nc.sync.dma_start(out=out[:, :], in_=outt[:])

    nc.compile()
    return nc


def _get_nc():
    if "nc" not in _CACHE:
        _CACHE["nc"] = _build()
    return _CACHE["nc"]


def _make_in_maps(tags: np.ndarray, joints: np.ndarray):
    tags = np.asarray(tags, dtype=np.float32).reshape(B, N)
    joints = np.asarray(joints, dtype=np.int32)
    boffs = (np.arange(BL, dtype=np.int32) * N)[:, None, None]
    in_maps = []
    for i in range(M):
        t = np.ascontiguousarray(tags[i * BL:(i + 1) * BL].reshape(BL * N, 1))
        sl = joints[i * BL:(i + 1) * BL]  # [BL, P, J, 2]
        ji = (sl[..., 0] + boffs).reshape(PART, J)
        jv = sl[..., 1].reshape(PART, J)
        jnt = np.ascontiguousarray(np.concatenate([ji, jv], axis=1))
        in_maps.append({"tags": t, "jnt": jnt})
    return in_maps


def _run(tags, joints, trace=False):
    from concourse.bass_utils import run_bass_kernel_spmd

    nc = _get_nc()
    in_maps = _make_in_maps(tags, joints)
    res = run_bass_kernel_spmd(
        nc, in_maps, core_ids=list(range(M)), trace=trace,
    )
    outs = [res.results[i]["out"] for i in range(M)]
    push = np.concatenate([o[:, 0] for o in outs]).astype(np.float32)
    pull = np.concatenate([o[:, 1] for o in outs]).astype(np.float32)
    return (push, pull), res.exec_time_ns


def kernel(tags, joints):
    (push, pull), _ = _run(tags, joints, trace=False)
    return push, pull
